# revision 1
# baseline (speedup 1.0000x reference)
"""Two-layer GAT (DGL GATConv-style) on 8 Trainium2 NeuronCores via Bass/Tile.

Strategy
--------
* Edges are sorted by destination on the host; each core owns a contiguous
  range of N/8 destination nodes and the edges pointing into it.
* Per layer, every core computes the full node-level projection table
  tab[n] = [h(n) in bf16 | el(n) f32 | er(n) f32]  (row = 272 bf16 = 544B)
  redundantly (layer 1 from the replicated input x, layer 2 from the
  all-gathered layer-1 activations), so edge gathers are core-local.
* Edge phase: for each window of 128 destination nodes, edges are processed
  in 128-edge tiles. Per-edge data is fetched with large batched indirect
  DMAs (row gather by src, plus a 16B er gather by dst). Scores
  ee = exp(leaky_relu(el[src]+er[dst])) are computed chunk-wide; the
  segment sums over destinations are done with a one-hot matmul
  (lhsT = onehot(dst_local) [128e x 128d], rhs = [h[src]*ee | ee]) that
  accumulates the whole window in PSUM. The epilogue divides by the summed
  ee (so no segment max / softmax shift is needed - scores are O(1)),
  adds bias, applies tanh+head-mean (layer 1) and writes the result.
* Between layers a single AllGather shares the (transposed, bf16) layer-1
  activations.

The mathematical identity used: alpha = ee/denom[dst] applied per edge
equals dividing the aggregated sum by denom once per destination.
exp(e - emax) / sum exp(e - emax) == exp(e) / sum exp(e) exactly in R.
"""

import math
import sys
from contextlib import ExitStack

import numpy as np

sys.path.insert(0, "/opt/trn_rl_repo")

import concourse.bass as bass  # noqa: E402
import concourse.mybir as mybir  # noqa: E402
from concourse.bass import IndirectOffsetOnAxis  # noqa: E402
from concourse.bass_utils import run_bass_kernel_spmd  # noqa: E402
from concourse.masks import make_identity  # noqa: E402
from concourse.tile import TileContext  # noqa: E402

BF16 = mybir.dt.bfloat16
F32 = mybir.dt.float32
I32 = mybir.dt.int32
NP_BF16 = mybir.dt.np(BF16)

AF = mybir.ActivationFunctionType
ALU = mybir.AluOpType

M_CORES = 8
NEG_SLOPE = 0.2
G_TILES = 32  # gather-chunk size in 128-edge tiles


# ----------------------------------------------------------------------------
# Host-side preprocessing
# ----------------------------------------------------------------------------
class Cfg:
    pass


def _ceil_div(a, b):
    return -(-a // b)


def _prepare(x, src, dst, W1, al1, ar1, b1, W2, al2, ar2, b2, m_cores=M_CORES):
    cfg = Cfg()
    N, F = x.shape
    E = src.shape[0]
    H = al1.shape[0]
    assert N % m_cores == 0
    npc = N // m_cores
    wn = _ceil_div(npc, 128)
    HF = H * F

    cfg.N, cfg.F, cfg.E, cfg.H, cfg.M = N, F, E, H, m_cores
    cfg.NPC, cfg.WN, cfg.HF = npc, wn, HF
    cfg.ROWC = HF + 4 * H  # bf16 cols: h | el(f32 bits) | er(f32 bits)
    cfg.MC = HF + H  # matmul rhs cols: scaled h | ee
    cfg.AUGC = HF + 2 * H  # node-matmul output cols: h | el | er

    # ---- edge partition: sort by dst, split by dst range, window by 128 ----
    order = np.argsort(dst, kind="stable")
    ss = src[order].astype(np.int64)
    ds = dst[order].astype(np.int64)
    core = ds // npc
    dl = ds % npc
    win = dl // 128
    dloc = (dl - win * 128).astype(np.float32)

    grp = core * wn + win  # non-decreasing
    counts = np.bincount(grp, minlength=m_cores * wn).reshape(m_cores, wn)
    tw = np.maximum(1, _ceil_div(counts.max(axis=0), 128))  # tiles per window
    ttot = int(tw.sum())
    base = np.zeros(wn + 1, np.int64)
    base[1:] = np.cumsum(tw * 128)
    starts = np.searchsorted(grp, np.arange(m_cores * wn))
    ends = np.searchsorted(grp, np.arange(m_cores * wn) + 1)

    soff = np.zeros((m_cores, 128, ttot), np.int32)
    doff = np.zeros((m_cores, 128, ttot), np.int32)
    dlocs = np.zeros((m_cores, 128, ttot), np.float32)
    for c in range(m_cores):
        s_src = np.zeros(ttot * 128, np.int64)
        s_dst = np.zeros(ttot * 128, np.int64)
        s_dlc = np.full(ttot * 128, -1.0, np.float32)
        for w in range(wn):
            s0, e0 = starts[c * wn + w], ends[c * wn + w]
            n = e0 - s0
            b0 = base[w]
            s_src[b0:b0 + n] = ss[s0:e0]
            s_dst[b0:b0 + n] = ds[s0:e0]
            s_dlc[b0:b0 + n] = dloc[s0:e0]
        soff[c] = s_src.reshape(ttot, 128).T
        doff[c] = s_dst.reshape(ttot, 128).T
        dlocs[c] = s_dlc.reshape(ttot, 128).T

    dwin = np.zeros((m_cores, 128, wn), np.int32)
    p_ar = np.arange(128)
    for c in range(m_cores):
        for w in range(wn):
            dw = min(128, npc - w * 128)
            dwin[c, :, w] = c * npc + w * 128 + np.minimum(p_ar, dw - 1)

    cfg.TW = [int(t) for t in tw]
    cfg.TTOT = ttot
    # tile -> window map and first/last flags
    win_of, first_t, last_t = [], [], []
    for w in range(wn):
        for i in range(cfg.TW[w]):
            win_of.append(w)
            first_t.append(i == 0)
            last_t.append(i == cfg.TW[w] - 1)
    cfg.win_of, cfg.first_t, cfg.last_t = win_of, first_t, last_t

    # ---- folded weights: el = x @ (W . al), appended to W ----
    def aug(Wm, al, ar):
        W64 = Wm.astype(np.float64).reshape(F, H, F)
        wal = np.einsum("khf,hf->kh", W64, al.astype(np.float64))
        war = np.einsum("khf,hf->kh", W64, ar.astype(np.float64))
        return np.concatenate(
            [Wm.astype(np.float64), wal, war], axis=1
        ).astype(NP_BF16)

    W1a = aug(W1, al1, ar1)
    W2a = aug(W2, al2, ar2)
    xT = np.ascontiguousarray(x.T).astype(NP_BF16)
    b1r = np.tile(b1.reshape(1, HF), (128, 1)).astype(np.float32)
    b2r = np.tile(b2.reshape(1, HF), (128, 1)).astype(np.float32)
    iota = np.tile(np.arange(128, dtype=np.float32), (128, 1))

    in_maps = []
    for c in range(m_cores):
        in_maps.append(
            dict(
                xT=xT, W1a=W1a, W2a=W2a, b1r=b1r, b2r=b2r, iota=iota,
                soff=np.ascontiguousarray(soff[c]),
                doff=np.ascontiguousarray(doff[c]),
                dloc=np.ascontiguousarray(dlocs[c]),
                dwin=np.ascontiguousarray(dwin[c]),
            )
        )
    return cfg, in_maps


# ----------------------------------------------------------------------------
# Bass program
# ----------------------------------------------------------------------------
def build_program(cfg):
    N, F, H, M = cfg.N, cfg.F, cfg.H, cfg.M
    HF, NPC, WN = cfg.HF, cfg.NPC, cfg.WN
    ROWC, MC, AUGC = cfg.ROWC, cfg.MC, cfg.AUGC

    nc = bass.Bass(num_devices=M)

    xT_d = nc.dram_tensor("xT", [F, N], BF16, kind="ExternalInput")
    W1a_d = nc.dram_tensor("W1a", [F, AUGC], BF16, kind="ExternalInput")
    W2a_d = nc.dram_tensor("W2a", [F, AUGC], BF16, kind="ExternalInput")
    b1r_d = nc.dram_tensor("b1r", [128, HF], F32, kind="ExternalInput")
    b2r_d = nc.dram_tensor("b2r", [128, HF], F32, kind="ExternalInput")
    iota_d = nc.dram_tensor("iota", [128, 128], F32, kind="ExternalInput")
    soff_d = nc.dram_tensor("soff", [128, cfg.TTOT], I32, kind="ExternalInput")
    doff_d = nc.dram_tensor("doff", [128, cfg.TTOT], I32, kind="ExternalInput")
    dloc_d = nc.dram_tensor("dloc", [128, cfg.TTOT], F32, kind="ExternalInput")
    dwin_d = nc.dram_tensor("dwin", [128, WN], I32, kind="ExternalInput")
    out_d = nc.dram_tensor("out", [NPC, F], F32, kind="ExternalOutput")
    dbg = getattr(cfg, "debug", False)
    if dbg:
        dtab_d = nc.dram_tensor("dtab", [N, ROWC], BF16, kind="ExternalOutput")
        drow_d = nc.dram_tensor(
            "drow", [128, G_TILES * ROWC], BF16, kind="ExternalOutput"
        )
        der_d = nc.dram_tensor(
            "der", [128, G_TILES * 8], BF16, kind="ExternalOutput"
        )
        dee_d = nc.dram_tensor(
            "dee", [128, G_TILES * 4], F32, kind="ExternalOutput"
        )

    tab1_d = nc.dram_tensor("tab1", [N, ROWC], BF16, kind="Internal")
    tab2_d = nc.dram_tensor("tab2", [N, ROWC], BF16, kind="Internal")
    h1Ts_d = nc.dram_tensor("h1Ts", [F, NPC], BF16, kind="Internal")
    h1Tf_d = nc.dram_tensor(
        "h1Tf", [M, F, NPC], BF16, kind="Internal", addr_space="Shared"
    )

    with ExitStack() as ctx:
        tc = ctx.enter_context(TileContext(nc))
        const = ctx.enter_context(tc.tile_pool(name="const", bufs=1))
        nxt_p = ctx.enter_context(tc.tile_pool(name="nxt", bufs=4))
        nhb_p = ctx.enter_context(tc.tile_pool(name="nhb", bufs=4))
        rows_p = ctx.enter_context(tc.tile_pool(name="rows", bufs=2))
        er_p = ctx.enter_context(tc.tile_pool(name="erp", bufs=4))
        off_p = ctx.enter_context(tc.tile_pool(name="off", bufs=2))
        sc_p = ctx.enter_context(tc.tile_pool(name="sc", bufs=8))
        m_p = ctx.enter_context(tc.tile_pool(name="m", bufs=6))
        oh_p = ctx.enter_context(tc.tile_pool(name="oh", bufs=8))
        ep_p = ctx.enter_context(tc.tile_pool(name="ep", bufs=2))
        ps_node = ctx.enter_context(tc.tile_pool(name="psn", bufs=3, space="PSUM"))
        ps_agg = ps_node
        ps_tr = ctx.enter_context(tc.tile_pool(name="pst", bufs=2, space="PSUM"))
        ps_er = ctx.enter_context(tc.tile_pool(name="pse", bufs=2, space="PSUM"))

        # constants
        W1_sb = const.tile([F, AUGC], BF16)
        nc.sync.dma_start(W1_sb[:], W1a_d[:, :])
        W2_sb = const.tile([F, AUGC], BF16)
        nc.sync.dma_start(W2_sb[:], W2a_d[:, :])
        b1_sb = const.tile([128, HF], F32)
        nc.sync.dma_start(b1_sb[:], b1r_d[:, :])
        b2_sb = const.tile([128, HF], F32)
        nc.sync.dma_start(b2_sb[:], b2r_d[:, :])
        iota_sb = const.tile([128, 128], F32)
        nc.sync.dma_start(iota_sb[:], iota_d[:, :])
        ident_sb = const.tile([128, 128], F32)
        make_identity(nc, ident_sb[:])
        identb_sb = const.tile([128, 128], BF16)
        nc.vector.tensor_copy(identb_sb[:], ident_sb[:])

        def node_tile(tab_d, W_sb, n0, cnt, lhsT_src_ap):
            """project one 128-node tile and write its table rows."""
            xt = nxt_p.tile([F, 128], BF16, tag="xt")
            nc.sync.dma_start(xt[:, :cnt], lhsT_src_ap)
            ps = ps_node.tile([128, AUGC], F32, tag="agg", name="psnode")
            nc.tensor.matmul(
                ps[:cnt, :], lhsT=xt[:, :cnt], rhs=W_sb[:], start=True, stop=True
            )
            hb = nhb_p.tile([128, HF], BF16, tag="hb")
            if (n0 // 128) % 2 == 0:
                nc.vector.tensor_copy(hb[:cnt, :], ps[:cnt, :HF])
            else:
                nc.scalar.activation(hb[:cnt, :], ps[:cnt, :HF], AF.Copy)
            elr = nhb_p.tile([128, 2 * H], F32, tag="elr")
            nc.vector.tensor_copy(elr[:cnt, :], ps[:cnt, HF:AUGC])
            nc.sync.dma_start(tab_d[n0:n0 + cnt, 0:HF], hb[:cnt, :])
            tabf = tab_d.bitcast(F32)
            fc = HF // 2  # f32 col where el starts
            nc.sync.dma_start(tabf[n0:n0 + cnt, fc:fc + 2 * H], elr[:cnt, :])

        def node_phase_l1():
            n0 = 0
            while n0 < N:
                cnt = min(128, N - n0)
                node_tile(tab1_d, W1_sb, n0, cnt, xT_d[:, n0:n0 + cnt])
                n0 += cnt

        def node_phase_l2():
            for c8 in range(M):
                j = 0
                while j < NPC:
                    cnt = min(128, NPC - j)
                    node_tile(
                        tab2_d, W2_sb, c8 * NPC + j, cnt,
                        h1Tf_d[c8, :, j:j + cnt],
                    )
                    j += cnt

        def epilogue(layer, w, psw):
            dw = min(128, NPC - w * 128)
            rec0 = ep_p.tile([128, H], F32, tag="rec0")
            nc.vector.tensor_scalar(
                out=rec0[:], in0=psw[:, HF:HF + H], scalar1=1e-30, scalar2=None,
                op0=ALU.add,
            )
            rec = ep_p.tile([128, H], F32, tag="rec")
            nc.vector.reciprocal(rec[:], rec0[:])
            o = ep_p.tile([128, HF], F32, tag="o")
            for hd in range(H):
                sl = slice(hd * F, (hd + 1) * F)
                if hd % 2 == 0:
                    nc.vector.tensor_scalar_mul(
                        o[:, sl], psw[:, sl], rec[:, hd:hd + 1]
                    )
                else:
                    nc.scalar.activation(
                        o[:, sl], psw[:, sl], AF.Copy, scale=rec[:, hd:hd + 1]
                    )
            o2 = ep_p.tile([128, HF], F32, tag="o2")
            b_sb = b1_sb if layer == 1 else b2_sb
            nc.vector.tensor_tensor(
                out=o2[:], in0=o[:], in1=b_sb[:], op=ALU.add
            )
            if layer == 1:
                o3 = ep_p.tile([128, HF], F32, tag="o3")
                nc.scalar.activation(o3[:], o2[:], AF.Tanh)
                src_t = o3
            else:
                src_t = o2
            t1 = ep_p.tile([128, F], F32, tag="t1")
            nc.vector.tensor_tensor(
                out=t1[:], in0=src_t[:, 0:F], in1=src_t[:, F:2 * F], op=ALU.add
            )
            t2 = ep_p.tile([128, F], F32, tag="t2")
            nc.vector.tensor_tensor(
                out=t2[:], in0=src_t[:, 2 * F:3 * F], in1=src_t[:, 3 * F:4 * F],
                op=ALU.add,
            )
            t3 = ep_p.tile([128, F], F32, tag="t3")
            nc.vector.tensor_tensor(out=t3[:], in0=t1[:], in1=t2[:], op=ALU.add)
            if layer == 1:
                hm = ep_p.tile([128, F], F32, tag="hm")
                nc.vector.tensor_scalar_mul(hm[:], t3[:], 1.0 / H)
                pst = ps_er.tile([128, 128], F32, tag="erp", name="pstr")[:F, :]
                nc.tensor.transpose(pst[:], hm[:], ident_sb[:])
                hT = ep_p.tile([F, 128], BF16, tag="hT")
                nc.vector.tensor_copy(hT[:], pst[:])
                nc.sync.dma_start(
                    h1Ts_d[:, w * 128:w * 128 + dw], hT[:, :dw]
                )
            else:
                om = ep_p.tile([128, F], F32, tag="om")
                nc.vector.tensor_scalar_mul(om[:], t3[:], 1.0 / H)
                nc.sync.dma_start(out_d[w * 128:w * 128 + dw, :], om[:dw, :])

        def edge_phase(layer, tab_d):
            cur_psum = {}
            cur_erwb = {}
            dwin_sb = off_p.tile([128, WN], I32, tag="dwin", name="dwin")
            nc.sync.dma_start(dwin_sb[:], dwin_d[:, :])
            g0 = 0
            while g0 < cfg.TTOT:
                gc = min(G_TILES, cfg.TTOT - g0)
                rows = rows_p.tile([128, G_TILES * ROWC], BF16, tag="rows")
                so = off_p.tile([128, G_TILES], I32, tag="so")
                dlt = off_p.tile([128, G_TILES], F32, tag="dl")
                nc.sync.dma_start(so[:, :gc], soff_d[:, g0:g0 + gc])
                nc.sync.dma_start(dlt[:, :gc], dloc_d[:, g0:g0 + gc])
                for t in range(gc):
                    gt = g0 + t
                    w = cfg.win_of[gt]
                    if not getattr(cfg, "skip_hg", False):
                        nc.gpsimd.indirect_dma_start(
                            out=rows[:, t * ROWC:(t + 1) * ROWC],
                            out_offset=None,
                            in_=tab_d[:, :],
                            in_offset=IndirectOffsetOnAxis(
                                ap=so[:, t:t + 1], axis=0
                            ),
                        )
                    if cfg.first_t[gt]:
                        erw = er_p.tile([128, 2 * H], BF16, tag="erw",
                                        name="erw")
                        nc.gpsimd.indirect_dma_start(
                            out=erw[:], out_offset=None, in_=tab_d[:, :],
                            in_offset=IndirectOffsetOnAxis(
                                ap=dwin_sb[:, w:w + 1], axis=0),
                            element_offset=HF + 2 * H,
                        )
                        erwb = er_p.tile([128, H], BF16, tag="erwb",
                                         name="erwb")
                        nc.vector.tensor_copy(erwb[:], erw[:].bitcast(F32))
                        cur_erwb[w] = erwb
                        cur_psum[w] = ps_agg.tile(
                            [128, MC], F32, tag="agg", name="aggps"
                        )
                    oh = oh_p.tile([128, 128], BF16, tag="oh", name="ohp")
                    nc.vector.tensor_scalar(
                        out=oh[:], in0=iota_sb[:], scalar1=dlt[:, t:t + 1],
                        scalar2=None, op0=ALU.is_equal,
                    )
                    otp = ps_tr.tile([128, 128], BF16, tag="otr", name="otp")
                    nc.tensor.transpose(otp[:], oh[:], identb_sb[:])
                    ots = oh_p.tile([128, 128], BF16, tag="ots", name="ots")
                    nc.vector.tensor_copy(ots[:], otp[:])
                    erp = ps_er.tile([128, H], F32, tag="erp", name="erp")
                    nc.tensor.matmul(
                        erp[:], lhsT=ots[:], rhs=cur_erwb[w][:],
                        start=True, stop=True,
                    )
                    el_v = rows[:, t * ROWC + HF:t * ROWC + HF + 2 * H]\
                        .bitcast(F32)
                    sc = sc_p.tile([128, H], F32, tag="sc", name="sc")
                    nc.vector.tensor_tensor(
                        out=sc[:], in0=el_v, in1=erp[:], op=ALU.add
                    )
                    sn = sc_p.tile([128, H], F32, tag="sn", name="sn")
                    nc.vector.tensor_scalar_mul(sn[:], sc[:], NEG_SLOPE)
                    lr = sc_p.tile([128, H], F32, tag="lr", name="lr")
                    nc.vector.tensor_tensor(
                        out=lr[:], in0=sc[:], in1=sn[:], op=ALU.max
                    )
                    ee = sc_p.tile([128, H], F32, tag="ee", name="ee")
                    nc.scalar.activation(ee[:], lr[:], AF.Exp)
                    m_t = m_p.tile([128, MC], BF16, tag="m", name="mt")
                    nc.vector.tensor_copy(m_t[:, HF:HF + H], ee[:])
                    h_sl = rows[:, t * ROWC:t * ROWC + HF]
                    for hd in range(H):
                        msl = m_t[:, hd * F:(hd + 1) * F]
                        hsl = h_sl[:, hd * F:(hd + 1) * F]
                        eesl = ee[:, hd:hd + 1]
                        if hd % 2 == 0:
                            nc.vector.tensor_scalar_mul(msl, hsl, eesl)
                        else:
                            nc.scalar.activation(
                                msl, hsl, AF.Copy, scale=eesl
                            )
                    nc.tensor.matmul(
                        cur_psum[w][:],
                        lhsT=oh[:],
                        rhs=m_t[:],
                        start=cfg.first_t[gt],
                        stop=cfg.last_t[gt],
                    )
                    if dbg and layer == 1 and gt == 0:
                        nc.sync.dma_start(drow_d[:, :ROWC], rows[:, :ROWC])
                        nc.sync.dma_start(dee_d[:, :H], ee[:, :H])
                    if cfg.last_t[gt]:
                        cur_erwb.pop(w)
                        epilogue(layer, w, cur_psum.pop(w)[:])
                g0 += gc

        node_phase_l1()
        if not getattr(cfg, "skip_edge", False):
            edge_phase(1, tab1_d)
        else:
            zz = ep_p.tile([F, 128], BF16, tag="hT")
            nc.gpsimd.memset(zz[:], 0.0)
            nc.sync.dma_start(h1Ts_d[:, 0:128], zz[:, 0:128])
        nc.gpsimd.collective_compute(
            "AllGather",
            ALU.bypass,
            replica_groups=[list(range(M))],
            ins=[h1Ts_d[:, :]],
            outs=[h1Tf_d[:, :, :]],
        )
        node_phase_l2()
        if not getattr(cfg, "skip_edge", False):
            edge_phase(2, tab2_d)
        else:
            zo = ep_p.tile([128, F], F32, tag="om")
            nc.gpsimd.memset(zo[:], 0.0)
            nc.sync.dma_start(out_d[0:128, :], zo[:])
        if dbg:
            nc.sync.dma_start(dtab_d[:, :], tab1_d[:, :])

    _cap_dma_waits(nc)
    return nc


def _cap_dma_waits(nc):
    """walrus' pseudo-instruction encodings hold only a couple of sync-wait
    commands (DMA DIRECT2D keeps 1 slot for itself), but Tile can emit more
    (slot WAR + WAW + HWDGE-ring wait). Hoist the excess onto same-engine
    NoOps placed just before the instruction."""
    import bass_rust

    skip = (
        mybir.InstEventSemaphore,
        mybir.InstAllEngineBarrier,
        mybir.InstHalt,
        mybir.InstBranchHint,
    )
    ctr = 0
    for f in nc.m.functions:
        for blk in f.blocks:
            out = []
            changed = False
            for ins in blk.instructions:
                si = ins.sync_info
                if isinstance(ins, skip) or si is None or not si.on_wait:
                    out.append(ins)
                    continue
                cap = 1
                if len(si.on_wait) > cap:
                    waits = list(si.on_wait)
                    extra, keep = waits[:-cap], waits[-cap:]
                    while extra:
                        take, extra = extra[:1], extra[1:]
                        ctr += 1
                        nop = mybir.InstNoOp(
                            name=f"I-waitcap-{ctr}", ins=[], outs=[]
                        )
                        nop.engine = ins.engine
                        nop.sync_info = bass_rust.SyncInfo(
                            on_wait=take, on_update=[]
                        )
                        out.append(nop)
                    ins.sync_info = bass_rust.SyncInfo(
                        on_wait=keep, on_update=list(si.on_update or [])
                    )
                    changed = True
                out.append(ins)
            if changed:
                blk.instructions = out


# ----------------------------------------------------------------------------
# Entry point
# ----------------------------------------------------------------------------
_CACHE = {}


def _run(inputs, trace=False):
    cfg, in_maps = _prepare(**inputs)
    key = (cfg.N, cfg.E, cfg.H, cfg.F, cfg.TTOT, tuple(cfg.TW))
    if key not in _CACHE:
        _CACHE[key] = build_program(cfg)
    nc = _CACHE[key]
    res = run_bass_kernel_spmd(
        nc, in_maps, core_ids=list(range(cfg.M)), trace=trace
    )
    shards = [res.results[c]["out"] for c in range(cfg.M)]
    out = np.concatenate(shards, axis=0).astype(np.float32)
    return out, res


def kernel(**inputs):
    out, _ = _run(inputs, trace=False)
    return out


def hw_time(inputs, iters=20):
    """Estimate per-execution device time: jit once, device-put inputs,
    then (a) sequential blocking calls, (b) pipelined queue of `iters`
    calls with one final block (hides per-call dispatch latency)."""
    import time

    import jax

    from concourse import bass2jax
    from concourse.bass2jax import _bass_exec_p, partition_id_tensor

    cfg, in_maps = _prepare(**inputs)
    key = (cfg.N, cfg.E, cfg.H, cfg.F, cfg.TTOT, tuple(cfg.TW))
    if key not in _CACHE:
        _CACHE[key] = build_program(cfg)
    nc = _CACHE[key]
    bass2jax.install_neuronx_cc_hook()

    partition_name = (
        nc.partition_id_tensor.name if nc.partition_id_tensor else None
    )
    in_names, out_names, out_avals, zero_outs = [], [], [], []
    for alloc in nc.m.functions[0].allocations:
        if not isinstance(alloc, mybir.MemoryLocationSet):
            continue
        name = alloc.memorylocations[0].name
        if alloc.kind == "ExternalInput":
            if name != partition_name:
                in_names.append(name)
        elif alloc.kind == "ExternalOutput":
            shape = tuple(alloc.tensor_shape)
            dtype = mybir.dt.np(alloc.dtype)
            out_avals.append(jax.core.ShapedArray(shape, dtype))
            out_names.append(name)
            zero_outs.append(np.zeros(shape, dtype))
    n_params = len(in_names)
    all_names = list(in_names) + out_names
    if partition_name is not None:
        all_names.append(partition_name)

    def _body(*args):
        operands = list(args)
        if partition_name is not None:
            operands.append(partition_id_tensor())
        outs = _bass_exec_p.bind(
            *operands,
            out_avals=tuple(out_avals),
            in_names=tuple(all_names),
            out_names=tuple(out_names),
            lowering_input_output_aliases=(),
            sim_require_finite=True,
            sim_require_nnan=True,
            nc=nc,
        )
        return tuple(outs)

    from jax.sharding import Mesh, PartitionSpec
    from jax.experimental.shard_map import shard_map

    M = cfg.M
    devices = jax.devices()[:M]
    mesh = Mesh(np.asarray(devices), ("core",))
    in_specs = (PartitionSpec("core"),) * (n_params + len(out_names))
    out_specs = (PartitionSpec("core"),) * len(out_names)
    fn = jax.jit(
        shard_map(
            _body, mesh=mesh, in_specs=in_specs, out_specs=out_specs,
            check_rep=False,
        ),
        keep_unused=True,
    )
    concat_in = [
        np.concatenate([np.asarray(in_maps[c][n]) for c in range(M)], axis=0)
        for n in in_names
    ]
    concat_zero = [
        np.zeros((M * z.shape[0], *z.shape[1:]), z.dtype) for z in zero_outs
    ]
    dev_in = [jax.device_put(a) for a in concat_in]
    dev_zero = [jax.device_put(a) for a in concat_zero]
    r = fn(*dev_in, *dev_zero)
    jax.block_until_ready(r)

    seq = []
    for _ in range(max(5, iters // 4)):
        t0 = time.perf_counter()
        r = fn(*dev_in, *dev_zero)
        jax.block_until_ready(r)
        seq.append(time.perf_counter() - t0)

    t0 = time.perf_counter()
    rs = [fn(*dev_in, *dev_zero) for _ in range(iters)]
    jax.block_until_ready(rs)
    piped = (time.perf_counter() - t0) / iters

    return dict(
        seq_min_s=float(np.min(seq)),
        seq_med_s=float(np.median(seq)),
        piped_avg_s=float(piped),
    )

